# revision 51
# baseline (speedup 1.0000x reference)
"""Trainium2 Bass kernel for nn_Attention1d (1x1-conv QKV attention block).

Full inputs in, full outputs out. Sharding: 8 cores = 4 batches x 2
head-groups (4 heads each). Each core: QKV projection for its heads,
attention with sim computed transposed ([keys, queries] so the AV matmul
needs no transpose), softmax denominator folded into the AV matmul via a
ones-row appended to V^T, partial output projection. The two head-group
partials per batch are summed on host (+ output bias).

Projection/attention matmuls run in bf16 (f32 PSUM accumulate); the
output projection runs in float32r (TF32-like, full PE rate).
"""

import sys

if "/opt/trn_rl_repo" not in sys.path:
    sys.path.insert(0, "/opt/trn_rl_repo")

import ml_dtypes
import numpy as np

import concourse.bass as bass
import concourse.bacc as bacc
import concourse.mybir as mybir
from concourse.tile import TileContext

HEADS = 8
D = 64  # head dim
C = 512  # model dim
L = 2048  # sequence length
B = 4  # batch
SCALE = D ** -0.5
HPC = 4  # heads per core
HID = HPC * D  # 256 channels per core
NCORES = 8

F32 = mybir.dt.float32
F32R = mybir.dt.float32r
BF16 = mybir.dt.bfloat16
EXP = mybir.ActivationFunctionType.Exp

_cache = {}


def build_module():
    nc = bacc.Bacc(None, target_bir_lowering=False)
    xb = nc.dram_tensor("xb", [C, L], BF16, kind="ExternalInput")
    wqkT = nc.dram_tensor("wqkT", [C, 2 * HID], BF16, kind="ExternalInput")
    wvT = nc.dram_tensor("wvT", [C, HID], BF16, kind="ExternalInput")
    woT = nc.dram_tensor("woT", [HID, C], F32R, kind="ExternalInput")
    outp = nc.dram_tensor("outp", [C, L], F32, kind="ExternalOutput")

    with TileContext(nc) as tc:
        with tc.tile_pool(name="wp", bufs=1) as wp, \
             tc.tile_pool(name="expp", bufs=14) as expp, \
             tc.tile_pool(name="small", bufs=4) as small, \
             tc.tile_pool(name="pp", bufs=2, space="PSUM") as pp, \
             tc.tile_pool(name="sp", bufs=2, space="PSUM") as sp:

            # ---- persistent SBUF tiles ----
            xb_sb = [wp.tile([128, L], BF16, name=f"xbs{i}", tag=f"xbs{i}")
                     for i in range(4)]
            wqk_sb = [wp.tile([128, 2 * HID], BF16, name=f"wqk{i}", tag=f"wqk{i}")
                      for i in range(4)]
            wv_sb = [wp.tile([128, HID], BF16, name=f"wv{i}", tag=f"wv{i}")
                     for i in range(4)]
            wo_sb = [wp.tile([128, C], F32R, name=f"wo{i}", tag=f"wo{i}")
                     for i in range(2)]
            # qk: 0: q heads01, 1: q heads23, 2: k heads01, 3: k heads23
            qk_sb = [wp.tile([128, L], BF16, name=f"qk{i}", tag=f"qk{i}")
                     for i in range(4)]
            # v^T augmented: 64 blocks (j-chunk, head) of 68 cols:
            # cols 0..63 = v^T, col 64 = 1.0 (denominator row of AV matmul)
            vT_sb = wp.tile([128, 64 * 68], BF16, name="vT", tag="vT")
            # normalized attention output, [c, i] layout, c-chunk per tile
            outn_sb = [wp.tile([128, L], F32R, name=f"on{i}", tag=f"on{i}")
                       for i in range(2)]

            # input loads: k-projection weights first, then x halves, so the
            # first QK-projection tile (and with it ScalarE) starts ASAP
            qeng = [nc.sync, nc.sync, nc.sync, nc.sync]
            for i in range(4):
                qeng[i].dma_start(out=wqk_sb[i][:, 256:384],
                                  in_=wqkT[i * 128:(i + 1) * 128, 256:384])
            for i in range(4):
                for half in range(2):
                    sl = slice(half * 1024, (half + 1) * 1024)
                    qeng[(i + half) % 4].dma_start(
                        out=xb_sb[i][:, sl],
                        in_=xb[i * 128:(i + 1) * 128, sl])
            for i in range(4):
                qeng[i].dma_start(out=wqk_sb[i][:, 0:256],
                                  in_=wqkT[i * 128:(i + 1) * 128, 0:256])
                qeng[(i + 1) % 4].dma_start(
                    out=wqk_sb[i][:, 384:512],
                    in_=wqkT[i * 128:(i + 1) * 128, 384:512])
            for i in range(4):
                qeng[i].dma_start(out=wv_sb[i], in_=wvT[i * 128:(i + 1) * 128, :])
            for i in range(2):
                qeng[i].dma_start(out=wo_sb[i], in_=woT[i * 128:(i + 1) * 128, :])

            # ones column of each vT block (softmax denominator row)
            vT3 = vT_sb.rearrange("p (b c) -> p b c", c=68)
            nc.vector.memset(vT3[:, :, 64:65], 1.0)

            # warm the ACT exp table while inputs load (the ~2.7us
            # ACT_TABLE_LOAD otherwise precedes the first real exp)
            warm = small.tile([1, 1], F32, name="warm", tag="warm")
            nc.vector.memset(warm, 0.0)
            nc.scalar.activation(warm, warm, EXP)

            # ---- Phase A: QK projection ----
            # qk[o, l] = sum_c wqkT[c, o] * x[c, l]
            def emit_A(oc, lt, pool=None):
                # prologue tiles borrow the (idle) sim pool's psum slots so
                # phase A isn't serialized on pp's two slots
                ps = (pool or pp).tile([128, 512], F32, name="ps512",
                                       tag="sim" if pool is sp else "ps512")
                for cc in range(4):
                    nc.tensor.matmul(
                        ps,
                        wqk_sb[cc][:, oc * 128:(oc + 1) * 128],
                        xb_sb[cc][:, lt * 512:(lt + 1) * 512],
                        start=(cc == 0), stop=(cc == 3))
                nc.vector.tensor_copy(
                    qk_sb[oc][:, lt * 512:(lt + 1) * 512], ps)

            # ---- Phase B: V^T projection ----
            # vT[l, o] = sum_c x[c, l] * wvT[c, o]
            def emit_B(lt):
                ps = pp.tile([128, HID], F32, name="ps512", tag="ps512")
                for cc in range(4):
                    nc.tensor.matmul(
                        ps,
                        xb_sb[cc][:, lt * 128:(lt + 1) * 128],
                        wv_sb[cc],
                        start=(cc == 0), stop=(cc == 3))
                vt_view = vT_sb[:, lt * 272:(lt + 1) * 272].rearrange(
                    "p (b c) -> p b c", c=68)
                ps_view = ps.rearrange("p (b c) -> p b c", c=64)
                nc.vector.tensor_copy(vt_view[:, :, 0:64], ps_view)

            # ---- attention: 8 pair-rounds R = ic*2 + hp ----
            # Each round: heads (2hp, 2hp+1) x i-chunk ic. Sim matmuls for the
            # head pair alternate partition halves (row groups 0-1 / 2-3) so
            # the PE runs them concurrently. AV matmuls are queued in a FIFO
            # and drained with a lag so the PE never waits on ScalarE's exp.
            av_fifo = []  # (R, et, off, h, jc, last)
            av_pair = {}  # R -> [av_even, av_odd] psum tiles

            op_queue = []  # pending out-proj steps (ic, oc)

            def emit_op_step():
                ic, oc = op_queue.pop(0)
                # out[o, i] = sum_c woT[c, o] * outn[c, i]
                ps = pp.tile([128, 512], F32, name="ps512", tag="ps512")
                for cci in range(2):
                    nc.tensor.matmul(
                        ps,
                        wo_sb[cci][:, oc * 128:(oc + 1) * 128],
                        outn_sb[cci][:, ic * 512:(ic + 1) * 512],
                        start=(cci == 0), stop=(cci == 1))
                ot = small.tile([128, 512], F32, name="ot", tag="ot")
                nc.vector.tensor_copy(ot, ps)
                nc.sync.dma_start(
                    out=outp[oc * 128:(oc + 1) * 128,
                             ic * 512:(ic + 1) * 512],
                    in_=ot)

            def finalize_head(R, hl):
                # av accumulation for (round R, head hl) is complete:
                # stage out of psum, normalize by the denominator row
                ic, hp = R // 2, R % 2
                h = 2 * hp + hl
                av = av_pair[R][hl]
                avs = small.tile([65, 512], F32, name="avs", tag="avs")
                nc.vector.tensor_copy(avs, av[0:65, :])
                den = small.tile([1, 512], F32, name="den", tag="den")
                nc.vector.tensor_copy(den, avs[64:65, :])
                recip = small.tile([1, 512], F32, name="recip", tag="recip")
                nc.vector.reciprocal_approx_fast(out=recip, in_=den)
                bc = small.tile([64, 512], F32, name="bc", tag="bc")
                nc.gpsimd.partition_broadcast(bc, recip)
                nc.vector.tensor_mul(
                    outn_sb[hp][(h % 2) * 64:(h % 2) * 64 + 64,
                                ic * 512:(ic + 1) * 512],
                    avs[0:64, :], bc)
                if hl == 1:
                    del av_pair[R]
                    if hp == 1:
                        op_queue.extend((ic, oc) for oc in range(4))

            def drain_one():
                R, et, off, h, jc, last = av_fifo.pop(0)
                if R not in av_pair:
                    av_pair[R] = [
                        pp.tile([128, 512], F32, name="ps512", tag="ps512")
                        for _ in range(2)]
                av = av_pair[R][h % 2]
                bi = jc * 4 + h
                nc.tensor.matmul(
                    av[0:65, :],
                    vT_sb[:, bi * 68:bi * 68 + 65],
                    et[:, off:off + 512],
                    start=(jc == 0), stop=(jc == 15))
                if last:
                    finalize_head(R, h % 2)

            # One full round (32 slots) of exp->av pipeline slack. Also
            # guarantees round 0 queues without draining: its AV matmuls
            # depend on phase B's vT, which is emitted after round 0.
            AV_LAG = 32

            prologue_work = []  # projection-tile thunks interleaved into R0

            def emit_round(R):
                # Sims are emitted in adjacent (h_even, h_odd) pairs so the PE
                # runs each pair concurrently (disjoint row groups). PSUM sim
                # tiles hold 3 slots; a pair may span two tiles; exp fires
                # when a tile fills.
                ic, hp = R // 2, R % 2
                qt = qk_sb[hp]
                kt = qk_sb[2 + hp]
                cur = {"sim": None, "et": None}

                def slot(s):
                    k = s % 3
                    if k == 0:
                        cur["sim"] = sp.tile([128, 1536], F32,
                                             name="sim", tag="sim")
                        cur["et"] = expp.tile([128, 1536], BF16,
                                              name="expt", tag="expt")
                    return cur["sim"], cur["et"], k * 512

                def fire_exp(size):
                    nc.scalar.activation(
                        cur["et"][:, 0:size * 512],
                        cur["sim"][:, 0:size * 512], EXP)

                for p in range(16):  # jc = p, pair (h_even, h_odd)
                    for hl in range(2):
                        s = 2 * p + hl
                        sim, et, off = slot(s)
                        hh = hl * 64
                        nc.tensor.matmul(
                            sim[:, off:off + 512],
                            kt[hh:hh + 64, p * 128:(p + 1) * 128],
                            qt[hh:hh + 64, ic * 512:(ic + 1) * 512],
                            start=True, stop=True)
                        if s % 3 == 2:
                            fire_exp(3)
                        av_fifo.append((R, et, off, 2 * hp + hl, p, s >= 30))
                    # remaining projection tiles fill PE slack during round 0
                    for _ in range(2):
                        if prologue_work:
                            prologue_work.pop(0)()
                    # round 7: taper the lag so the post-exp PE tail is short
                    lag = AV_LAG if R < 7 else max(6, AV_LAG - 2 * p)
                    while len(av_fifo) > lag:
                        drain_one()
                        drain_one()
                    if op_queue:
                        emit_op_step()
                fire_exp(2)  # slots 30,31 fill 2 of the last tile's 3

            # --- emission schedule ---
            # k-projection for heads 0,1 first, then one q tile, so round 0's
            # sims (and ScalarE) start as early as possible.
            for lt in range(4):
                emit_A(2, lt, pool=sp if lt % 2 == 0 else None)
            emit_A(0, 0, pool=sp)
            import functools
            for lt in range(1, 4):
                prologue_work.append(functools.partial(emit_A, 0, lt))
            for lt in range(4):
                prologue_work.append(functools.partial(emit_A, 3, lt))
            for lt in range(4):
                prologue_work.append(functools.partial(emit_A, 1, lt))
            for lt in range(16):
                prologue_work.append(functools.partial(emit_B, lt))
            emit_round(0)
            while prologue_work:
                prologue_work.pop(0)()
            for R in range(1, 8):
                emit_round(R)
            while av_fifo:
                drain_one()
            while op_queue:
                emit_op_step()

    nc.compile()
    return nc


def shard_inputs(x, w_qkv, w_out):
    """Build the 8 per-core input maps (numpy float32)."""
    wq = w_qkv[0:C]
    wk = w_qkv[C:2 * C]
    wv = w_qkv[2 * C:3 * C]
    in_maps = []
    for c in range(NCORES):
        b, g = c // 2, c % 2
        sl = slice(g * HID, (g + 1) * HID)
        wqkT = np.ascontiguousarray(
            np.concatenate([wq[sl] * SCALE, wk[sl]], axis=0).T)
        wvT = np.ascontiguousarray(wv[sl].T)
        woT = np.ascontiguousarray(w_out[:, sl].T)
        in_maps.append({
            "xb": np.ascontiguousarray(x[b]).astype(ml_dtypes.bfloat16),
            "wqkT": wqkT.astype(ml_dtypes.bfloat16),
            "wvT": wvT.astype(ml_dtypes.bfloat16),
            "woT": woT.astype(np.float32),
        })
    return in_maps


def run(inputs, trace=False, trace_kwargs=None):
    """Compile (cached) + run on the 8 cores. Returns (out, BassKernelResults)."""
    from concourse.bass_utils import run_bass_kernel_spmd

    if "nc" not in _cache:
        _cache["nc"] = build_module()
    nc = _cache["nc"]

    x = np.asarray(inputs["x"], dtype=np.float32)
    w_qkv = np.asarray(inputs["w_qkv"], dtype=np.float32)
    w_out = np.asarray(inputs["w_out"], dtype=np.float32)
    b_out = np.asarray(inputs["b_out"], dtype=np.float32)

    in_maps = shard_inputs(x, w_qkv, w_out)
    kwargs = dict(trace_kwargs or {})
    res = run_bass_kernel_spmd(
        nc, in_maps, core_ids=list(range(NCORES)), trace=trace, **kwargs)

    out = np.empty((B, C, L), dtype=np.float32)
    for b in range(B):
        out[b] = (res.results[2 * b]["outp"] + res.results[2 * b + 1]["outp"]
                  + b_out[:, None])
    return out, res


def kernel(**inputs):
    out, _ = run(inputs, trace=False)
    return out


# revision 54
# speedup vs baseline: 1.1760x; 1.1760x over previous
"""Trainium2 Bass kernel for nn_Attention1d (1x1-conv QKV attention block).

Full inputs in, full outputs out. Sharding: 8 cores = 4 batches x 2
head-groups (4 heads each). Each core: QKV projection for its heads,
attention with sim computed transposed ([keys, queries] so the AV matmul
needs no transpose), softmax denominator folded into the AV matmul via a
ones-row appended to V^T, partial output projection. The two head-group
partials per batch are summed on host (+ output bias).

Projection/attention matmuls run in bf16 (f32 PSUM accumulate); the
output projection runs in float32r (TF32-like, full PE rate).
"""

import sys

if "/opt/trn_rl_repo" not in sys.path:
    sys.path.insert(0, "/opt/trn_rl_repo")

import ml_dtypes
import numpy as np

import concourse.bass as bass
import concourse.bacc as bacc
import concourse.mybir as mybir
from concourse.tile import TileContext

HEADS = 8
D = 64  # head dim
C = 512  # model dim
L = 2048  # sequence length
B = 4  # batch
SCALE = D ** -0.5
HPC = 4  # heads per core
HID = HPC * D  # 256 channels per core
NCORES = 8

F32 = mybir.dt.float32
F32R = mybir.dt.float32r
BF16 = mybir.dt.bfloat16
EXP = mybir.ActivationFunctionType.Exp

_cache = {}


def build_module():
    nc = bacc.Bacc(None, target_bir_lowering=False)
    xb = nc.dram_tensor("xb", [C, L], BF16, kind="ExternalInput")
    wqkT = nc.dram_tensor("wqkT", [C, 2 * HID], BF16, kind="ExternalInput")
    wvT = nc.dram_tensor("wvT", [C, HID], BF16, kind="ExternalInput")
    woT = nc.dram_tensor("woT", [HID, C], F32R, kind="ExternalInput")
    outp = nc.dram_tensor("outp", [C, L], F32, kind="ExternalOutput")

    with TileContext(nc) as tc:
        with tc.tile_pool(name="wp", bufs=1) as wp, \
             tc.tile_pool(name="expp", bufs=14) as expp, \
             tc.tile_pool(name="small", bufs=4) as small, \
             tc.tile_pool(name="pp", bufs=2, space="PSUM") as pp, \
             tc.tile_pool(name="sp", bufs=2, space="PSUM") as sp:

            # ---- persistent SBUF tiles ----
            xb_sb = [wp.tile([128, L], BF16, name=f"xbs{i}", tag=f"xbs{i}")
                     for i in range(4)]
            wqk_sb = [wp.tile([128, 2 * HID], BF16, name=f"wqk{i}", tag=f"wqk{i}")
                      for i in range(4)]
            wv_sb = [wp.tile([128, HID], BF16, name=f"wv{i}", tag=f"wv{i}")
                     for i in range(4)]
            wo_sb = [wp.tile([128, C], F32R, name=f"wo{i}", tag=f"wo{i}")
                     for i in range(2)]
            # qk: 0: q heads01, 1: q heads23, 2: k heads01, 3: k heads23
            qk_sb = [wp.tile([128, L], BF16, name=f"qk{i}", tag=f"qk{i}")
                     for i in range(4)]
            # v^T augmented: 64 blocks (j-chunk, head) of 68 cols:
            # cols 0..63 = v^T, col 64 = 1.0 (denominator row of AV matmul)
            vT_sb = wp.tile([128, 64 * 68], BF16, name="vT", tag="vT")
            # normalized attention output, [c, i] layout, c-chunk per tile
            outn_sb = [wp.tile([128, L], F32R, name=f"on{i}", tag=f"on{i}")
                       for i in range(2)]

            # input loads: k-projection weights first, then x halves, so the
            # first QK-projection tile (and with it ScalarE) starts ASAP
            qeng = [nc.sync, nc.sync, nc.sync, nc.sync]
            # round 0 needs k01 (cols 256:384) and q01 (cols 0:128) first
            for i in range(4):
                qeng[i].dma_start(out=wqk_sb[i][:, 256:384],
                                  in_=wqkT[i * 128:(i + 1) * 128, 256:384])
            for i in range(4):
                qeng[i].dma_start(out=xb_sb[i][:, 0:1024],
                                  in_=xb[i * 128:(i + 1) * 128, 0:1024])
            for i in range(4):
                qeng[i].dma_start(out=wqk_sb[i][:, 0:128],
                                  in_=wqkT[i * 128:(i + 1) * 128, 0:128])
            for i in range(4):
                qeng[i].dma_start(out=xb_sb[i][:, 1024:2048],
                                  in_=xb[i * 128:(i + 1) * 128, 1024:2048])
            for i in range(4):
                qeng[i].dma_start(out=wqk_sb[i][:, 128:256],
                                  in_=wqkT[i * 128:(i + 1) * 128, 128:256])
                qeng[(i + 1) % 4].dma_start(
                    out=wqk_sb[i][:, 384:512],
                    in_=wqkT[i * 128:(i + 1) * 128, 384:512])
            for i in range(4):
                qeng[i].dma_start(out=wv_sb[i], in_=wvT[i * 128:(i + 1) * 128, :])
            for i in range(2):
                qeng[i].dma_start(out=wo_sb[i], in_=woT[i * 128:(i + 1) * 128, :])

            # ones column of each vT block (softmax denominator row)
            vT3 = vT_sb.rearrange("p (b c) -> p b c", c=68)
            nc.vector.memset(vT3[:, :, 64:65], 1.0)

            # warm the ACT exp table while inputs load (the ~2.7us
            # ACT_TABLE_LOAD otherwise precedes the first real exp)
            warm = small.tile([1, 1], F32, name="warm", tag="warm")
            nc.vector.memset(warm, 0.0)
            nc.scalar.activation(warm, warm, EXP)

            # ---- Phase A: QK projection ----
            # qk[o, l] = sum_c wqkT[c, o] * x[c, l]
            def emit_A(oc, lt, pool=None):
                # prologue tiles borrow the (idle) sim pool's psum slots so
                # phase A isn't serialized on pp's two slots
                ps = (pool or pp).tile([128, 512], F32, name="ps512",
                                       tag="sim" if pool is sp else "ps512")
                for cc in range(4):
                    nc.tensor.matmul(
                        ps,
                        wqk_sb[cc][:, oc * 128:(oc + 1) * 128],
                        xb_sb[cc][:, lt * 512:(lt + 1) * 512],
                        start=(cc == 0), stop=(cc == 3))
                nc.vector.tensor_copy(
                    qk_sb[oc][:, lt * 512:(lt + 1) * 512], ps)

            # ---- Phase B: V^T projection ----
            # vT[l, o] = sum_c x[c, l] * wvT[c, o]
            def emit_B(lt):
                ps = pp.tile([128, HID], F32, name="ps512", tag="ps512")
                for cc in range(4):
                    nc.tensor.matmul(
                        ps,
                        xb_sb[cc][:, lt * 128:(lt + 1) * 128],
                        wv_sb[cc],
                        start=(cc == 0), stop=(cc == 3))
                vt_view = vT_sb[:, lt * 272:(lt + 1) * 272].rearrange(
                    "p (b c) -> p b c", c=68)
                ps_view = ps.rearrange("p (b c) -> p b c", c=64)
                nc.vector.tensor_copy(vt_view[:, :, 0:64], ps_view)

            # ---- attention: 8 pair-rounds R = ic*2 + hp ----
            # Each round: heads (2hp, 2hp+1) x i-chunk ic. Sim matmuls for the
            # head pair alternate partition halves (row groups 0-1 / 2-3) so
            # the PE runs them concurrently. AV matmuls are queued in a FIFO
            # and drained with a lag so the PE never waits on ScalarE's exp.
            av_fifo = []  # (R, et, off, h, jc, last)
            av_pair = {}  # R -> [av_even, av_odd] psum tiles

            op_queue = []  # pending out-proj steps (ic, oc)

            def emit_op_step():
                ic, oc = op_queue.pop(0)
                # out[o, i] = sum_c woT[c, o] * outn[c, i]
                ps = pp.tile([128, 512], F32, name="ps512", tag="ps512")
                for cci in range(2):
                    nc.tensor.matmul(
                        ps,
                        wo_sb[cci][:, oc * 128:(oc + 1) * 128],
                        outn_sb[cci][:, ic * 512:(ic + 1) * 512],
                        start=(cci == 0), stop=(cci == 1))
                ot = small.tile([128, 512], F32, name="ot", tag="ot")
                nc.vector.tensor_copy(ot, ps)
                nc.sync.dma_start(
                    out=outp[oc * 128:(oc + 1) * 128,
                             ic * 512:(ic + 1) * 512],
                    in_=ot)

            def finalize_head(R, hl):
                # av accumulation for (round R, head hl) is complete:
                # stage out of psum, normalize by the denominator row
                ic, hp = R // 2, R % 2
                h = 2 * hp + hl
                av = av_pair[R][hl]
                avs = small.tile([65, 512], F32, name="avs", tag="avs")
                nc.vector.tensor_copy(avs, av[0:65, :])
                den = small.tile([1, 512], F32, name="den", tag="den")
                nc.vector.tensor_copy(den, avs[64:65, :])
                recip = small.tile([1, 512], F32, name="recip", tag="recip")
                nc.vector.reciprocal_approx_fast(out=recip, in_=den)
                bc = small.tile([64, 512], F32, name="bc", tag="bc")
                nc.gpsimd.partition_broadcast(bc, recip)
                nc.vector.tensor_mul(
                    outn_sb[hp][(h % 2) * 64:(h % 2) * 64 + 64,
                                ic * 512:(ic + 1) * 512],
                    avs[0:64, :], bc)
                if hl == 1:
                    del av_pair[R]
                    if hp == 1:
                        op_queue.extend((ic, oc) for oc in range(4))

            def drain_one():
                R, et, off, h, jc, last = av_fifo.pop(0)
                if R not in av_pair:
                    av_pair[R] = [
                        pp.tile([128, 512], F32, name="ps512", tag="ps512")
                        for _ in range(2)]
                av = av_pair[R][h % 2]
                bi = jc * 4 + h
                nc.tensor.matmul(
                    av[0:65, :],
                    vT_sb[:, bi * 68:bi * 68 + 65],
                    et[:, off:off + 512],
                    start=(jc == 0), stop=(jc == 15))
                if last:
                    finalize_head(R, h % 2)

            # One full round (32 slots) of exp->av pipeline slack. Also
            # guarantees round 0 queues without draining: its AV matmuls
            # depend on phase B's vT, which is emitted after round 0.
            AV_LAG = 32

            prologue_work = []  # projection-tile thunks interleaved into R0

            def emit_round(R):
                # Sims are emitted in adjacent (h_even, h_odd) pairs so the PE
                # runs each pair concurrently (disjoint row groups). PSUM sim
                # tiles hold 3 slots; a pair may span two tiles; exp fires
                # when a tile fills.
                ic, hp = R // 2, R % 2
                qt = qk_sb[hp]
                kt = qk_sb[2 + hp]
                cur = {"sim": None, "et": None}

                def slot(s):
                    k = s % 3
                    if k == 0:
                        cur["sim"] = sp.tile([128, 1536], F32,
                                             name="sim", tag="sim")
                        cur["et"] = expp.tile([128, 1536], BF16,
                                              name="expt", tag="expt")
                    return cur["sim"], cur["et"], k * 512

                def fire_exp(size):
                    nc.scalar.activation(
                        cur["et"][:, 0:size * 512],
                        cur["sim"][:, 0:size * 512], EXP)

                for p in range(16):  # jc = p, pair (h_even, h_odd)
                    for hl in range(2):
                        s = 2 * p + hl
                        sim, et, off = slot(s)
                        hh = hl * 64
                        nc.tensor.matmul(
                            sim[:, off:off + 512],
                            kt[hh:hh + 64, p * 128:(p + 1) * 128],
                            qt[hh:hh + 64, ic * 512:(ic + 1) * 512],
                            start=True, stop=True)
                        if s % 3 == 2:
                            fire_exp(3)
                        av_fifo.append((R, et, off, 2 * hp + hl, p, s >= 30))
                    # remaining projection tiles fill PE slack during round 0
                    for _ in range(2):
                        if prologue_work:
                            prologue_work.pop(0)()
                    # round 7: taper the lag so the post-exp PE tail is short
                    lag = AV_LAG if R < 7 else max(6, AV_LAG - 2 * p)
                    while len(av_fifo) > lag:
                        drain_one()
                        drain_one()
                    if op_queue:
                        emit_op_step()
                fire_exp(2)  # slots 30,31 fill 2 of the last tile's 3

            # --- emission schedule ---
            # k-projection for heads 0,1 first, then one q tile, so round 0's
            # sims (and ScalarE) start as early as possible.
            for lt in range(4):
                emit_A(2, lt, pool=sp if lt % 2 == 0 else None)
            emit_A(0, 0, pool=sp)
            import functools
            for lt in range(1, 4):
                prologue_work.append(functools.partial(emit_A, 0, lt))
            for lt in range(4):
                prologue_work.append(functools.partial(emit_A, 3, lt))
            for lt in range(4):
                prologue_work.append(functools.partial(emit_A, 1, lt))
            for lt in range(16):
                prologue_work.append(functools.partial(emit_B, lt))
            emit_round(0)
            while prologue_work:
                prologue_work.pop(0)()
            for R in range(1, 8):
                emit_round(R)
            while av_fifo:
                drain_one()
            while op_queue:
                emit_op_step()

    nc.compile()
    return nc


def shard_inputs(x, w_qkv, w_out):
    """Build the 8 per-core input maps (numpy float32)."""
    wq = w_qkv[0:C]
    wk = w_qkv[C:2 * C]
    wv = w_qkv[2 * C:3 * C]
    in_maps = []
    for c in range(NCORES):
        b, g = c // 2, c % 2
        sl = slice(g * HID, (g + 1) * HID)
        wqkT = np.ascontiguousarray(
            np.concatenate([wq[sl] * SCALE, wk[sl]], axis=0).T)
        wvT = np.ascontiguousarray(wv[sl].T)
        woT = np.ascontiguousarray(w_out[:, sl].T)
        in_maps.append({
            "xb": np.ascontiguousarray(x[b]).astype(ml_dtypes.bfloat16),
            "wqkT": wqkT.astype(ml_dtypes.bfloat16),
            "wvT": wvT.astype(ml_dtypes.bfloat16),
            "woT": woT.astype(np.float32),
        })
    return in_maps


def run(inputs, trace=False, trace_kwargs=None):
    """Compile (cached) + run on the 8 cores. Returns (out, BassKernelResults)."""
    from concourse.bass_utils import run_bass_kernel_spmd

    if "nc" not in _cache:
        _cache["nc"] = build_module()
    nc = _cache["nc"]

    x = np.asarray(inputs["x"], dtype=np.float32)
    w_qkv = np.asarray(inputs["w_qkv"], dtype=np.float32)
    w_out = np.asarray(inputs["w_out"], dtype=np.float32)
    b_out = np.asarray(inputs["b_out"], dtype=np.float32)

    in_maps = shard_inputs(x, w_qkv, w_out)
    kwargs = dict(trace_kwargs or {})
    res = run_bass_kernel_spmd(
        nc, in_maps, core_ids=list(range(NCORES)), trace=trace, **kwargs)

    out = np.empty((B, C, L), dtype=np.float32)
    for b in range(B):
        out[b] = (res.results[2 * b]["outp"] + res.results[2 * b + 1]["outp"]
                  + b_out[:, None])
    return out, res


def kernel(**inputs):
    out, _ = run(inputs, trace=False)
    return out
